# revision 25
# baseline (speedup 1.0000x reference)
"""Trainium2 Bass kernel for the BDH recurrent block (B=8, T=256, d=256, n=1024).

Key reformulation: the scan input v_prev is the *embedding* at each step (the
output v_star is never fed back), so the only recurrences are

  x_t = l1norm(0.97 * x_{t-1} + relu(emb_t @ Dx.T))          (elementwise, n)
  rho_t = 0.97 * rho_{t-1} + ln(emb_t) (x) x_t               (rank-1, d*n)

Both have closed forms:
  x_t  = sum_s C[t,s] * U_s           with U = relu(emb @ Dx.T)  and
         C[t,s] = 0.97^{t-s} / prod_{r=s..t} b_r,  b_r = sum(U_r) + 0.97*[r>0]
         (b_0 = sum(U_0)), computed in log space via a cumulative sum.
  a*_t = rho_{t-1} x_t = sum_{s<t} 0.97^{t-1-s} (x_s . x_t) ln(emb_s)
       = ((X X^T) o D) @ ln(emb)     -- decay-masked attention.

So the whole T-step scan becomes a handful of dense matmuls, one sample per
NeuronCore (data-parallel over B=8 across 8 cores, weights replicated).

v2 structural optimizations over the first working version:
 *  ln(a*) is never computed.  W rows are exactly zero-mean, so a* rows are
    zero-mean (mu_t = 0), and the per-row 1/sigma of ln(a*) commutes with relu
    and is absorbed by the final row-wise layernorm up to an exact eps
    correction:  out_t = (z - zbar) * rsqrt(var(z) + eps*(va_t + eps)) with
    va_t = var_d(a*_t).  va_t is computed off the critical path from squared
    A^T columns via tiny ones-matmuls.  This deletes the LN(A) chain and all
    four PE transposes: A^T = W^T @ (G o D) is computed directly.
 *  The causal -1e30 mask is accumulated into the p-broadcast PSUM banks via
    constant matmuls (strict-triangular x negated-identity), so CT is a single
    Exp activation per half reading PSUM with bias=q, no DVE op in between,
    and the maskCT DMA disappears.
 *  PE warm-up: the PE clock is HAM-gated at 1.2 GHz until it has been busy
    ~3.4us, and re-throttles after ~3.4us idle.  Junk matmuls on a zeroed tile
    cover the initial DMA wait, and low-priority filler matmuls let the
    scheduler plug remaining idle windows, keeping the array at 2.4 GHz.
 *  Critical input DMAs are split finer across both HWDGE queues and emb is
    loaded in bf16.
"""

import numpy as np
import ml_dtypes

import concourse.bass as bass
import concourse.tile as tile
from concourse import bacc, mybir
from concourse.bass_utils import run_bass_kernel_spmd
from concourse.tile_rust import add_dep_helper
from concourse.hw_specs import get_activation_tables

B, T, D, N = 8, 256, 256, 1024
P = 128  # partitions
LN_EPS = 1e-5
DECAY = 0.97
NEGBIG = -1e30
F32 = mybir.dt.float32
BF16 = mybir.dt.bfloat16
AF = mybir.ActivationFunctionType
ALU = mybir.AluOpType

N_JUNK_START = 6   # warm-up matmuls (free dim 256) before the first U matmul
N_JUNK_MID = 7     # filler matmuls bridging the log-cumsum phase


def _build_nc():
    nc = bacc.Bacc(enable_partition_id=False)

    # dram tensors.  critXa = [embT half X | DxT half X cols 0:512],
    # critXb = [DxT half X cols 512:1024] -- split so the first U matmuls
    # start as soon as the first 192KB lands.
    d_c0a = nc.dram_tensor("c0a", [P, T + 512], BF16, kind="ExternalInput")
    d_c0b = nc.dram_tensor("c0b", [P, 512], BF16, kind="ExternalInput")
    d_c1a = nc.dram_tensor("c1a", [P, T + 512], BF16, kind="ExternalInput")
    d_c1b = nc.dram_tensor("c1b", [P, 512], BF16, kind="ExternalInput")
    d_sc = nc.dram_tensor("sc", [T, 4], F32, kind="ExternalInput")
    # consts = [tri_strict(128) | ones(128) | cumA0(256) | cumA1(256) |
    #           negI0(256) | negI1(256)]  (bf16, [128, 1280])
    d_consts = nc.dram_tensor("consts", [P, 1536], BF16, kind="ExternalInput")
    d_emb = nc.dram_tensor("emb", [T, D], BF16, kind="ExternalInput")
    # late{k} = [DupT half k | DyT half k]
    d_late0 = nc.dram_tensor("late0", [P, T + N], BF16, kind="ExternalInput")
    d_late1 = nc.dram_tensor("late1", [P, T + N], BF16, kind="ExternalInput")
    # ET packed host-side as [p, k, d] -> [128, 2048]
    d_ET = nc.dram_tensor("ET", [P, 8 * D], BF16, kind="ExternalInput")
    d_out = nc.dram_tensor("out", [T, D], BF16, kind="ExternalOutput")

    # One ACT table set containing every function we use
    # (relu/ln/exp/copy/identity) so the compiler never swaps tables.
    act_sets = list(get_activation_tables(nc.m.arch))
    combined_set_id = act_sets.index("natural_log_exp_and_others")

    with tile.TileContext(nc) as tc:
        with (
            tc.tile_pool(name="consts", bufs=1) as cp,
            tc.tile_pool(name="work", bufs=1) as wp,
            tc.tile_pool(name="ps512", bufs=3, space="PSUM") as ps512,
            tc.tile_pool(name="ps256", bufs=3, space="PSUM") as ps256,
            tc.tile_pool(name="ps_junk", bufs=1, space="PSUM") as psj,
            tc.tile_pool(name="ps_small", bufs=1, space="PSUM") as pss,
        ):
            # ---- memset-built small consts FIRST: memsets don't wait for
            # the DMA ordering barrier, so the junk tile is ready ~6us and
            # the PE warm-up starts as early as possible.
            junk_sb = cp.tile([P, 256], BF16, tag="junk", name="junk")
            nc.gpsimd.memset(junk_sb[:], 0.0)
            ones2 = cp.tile([2, P], BF16, tag="ones2", name="ones2")
            nc.gpsimd.memset(ones2[:], 1.0)
            onescol = cp.tile([P, 1], BF16, tag="onescol", name="onescol")
            nc.gpsimd.memset(onescol[:], 1.0)
            zero_col = cp.tile([P, 1], F32, tag="zero_col", name="zero_col")
            nc.gpsimd.memset(zero_col[:], 0.0)
            eps_col = cp.tile([P, 1], F32, tag="eps_col", name="eps_col")
            nc.gpsimd.memset(eps_col[:], LN_EPS)

            # ---- loads: issue order == need order, both queues -------------
            c0a = cp.tile([P, T + 512], BF16, tag="c0a", name="c0a")
            nc.sync.dma_start(c0a[:], d_c0a[:, :])
            c1a = cp.tile([P, T + 512], BF16, tag="c1a", name="c1a")
            nc.scalar.dma_start(c1a[:], d_c1a[:, :])

            # act table load on scalar right after its critical dma issue
            nc.scalar.add_instruction(mybir.InstLoadActFuncSet(
                name=nc.get_next_instruction_name(),
                act_func_set_id=combined_set_id, ins=[], outs=[]))

            c0b = cp.tile([P, 512], BF16, tag="c0b", name="c0b")
            nc.sync.dma_start(c0b[:], d_c0b[:, :])
            c1b = cp.tile([P, 512], BF16, tag="c1b", name="c1b")
            nc.scalar.dma_start(c1b[:], d_c1b[:, :])

            embT_s = [c0a[:, 0:T], c1a[:, 0:T]]
            # DxT column-halves: ch0 in critXa, ch1 in critXb
            DxT_a = [c0a[:, T:T + 512], c1a[:, T:T + 512]]
            DxT_b = [c0b[:], c1b[:]]

            sc_t = cp.tile([P, 2, 4], F32, tag="sc", name="sc")
            nc.gpsimd.dma_start(sc_t[:], d_sc.rearrange("(m p) c -> p m c", p=P))

            consts = cp.tile([P, 1536], BF16, tag="consts", name="consts")
            tri_st = consts[:, 0:P]          # strict upper tri [k < c]
            ones_t = consts[:, P:2 * P]
            cumA = [consts[:, 256:512], consts[:, 512:768]]
            negI = [consts[:, 768:1024], consts[:, 1024:1280]]
            pk_t = consts[0:2, 1280:1536]

            emb_s = [cp.tile([P, D], BF16, tag=f"emb{k}", name=f"emb{k}")
                     for k in range(2)]
            late = [cp.tile([P, T + N], BF16, tag=f"late{k}", name=f"late{k}")
                    for k in range(2)]
            DupT_s = [t[:, 0:T] for t in late]
            DyT_s = [t[:, T:T + N] for t in late]
            et_big = cp.tile([P, 8, D], BF16, tag="et_big", name="et_big")
            ET_s = [et_big[:, k, :] for k in range(8)]

            # ---- PE warm-up: junk matmuls covering the initial DMA wait ----
            for j in range(N_JUNK_START):
                jp = psj.tile([P, 512], F32, tag="junkps", name=f"jstart{j}")
                nc.tensor.matmul(jp[:, 0:256], junk_sb[:, 0:P],
                                 junk_sb[:, 0:256], start=True, stop=True)

            # ---- U = relu(emb @ Dx.T), row sums a ---------------------------
            U_s = [wp.tile([P, N], BF16, tag=f"U{m}", name=f"U{m}")
                   for m in range(2)]
            reduce_insts = []
            apart2 = wp.tile([P, 2, 2], F32, tag="apart2", name="apart2")
            for ch in range(2):           # ch0 first: both row-sum halves asap
                for mt in range(2):
                    pu = ps512.tile([P, 512], F32, tag="pu", name="pu")
                    for k in range(2):
                        dx = DxT_a[k] if ch == 0 else DxT_b[k]
                        nc.tensor.matmul(
                            pu[:], embT_s[k][:, mt * P:(mt + 1) * P],
                            dx[:], start=(k == 0), stop=(k == 1))
                    with tc.high_priority():
                        if ch == 0 or mt == 1:
                            red = nc.vector.tensor_scalar(
                                U_s[mt][:, ch * 512:(ch + 1) * 512], pu[:],
                                0.0, 0.0, op0=ALU.max, op1=ALU.add,
                                accum_out=apart2[:, ch, mt:mt + 1])
                        else:
                            red = nc.scalar.activation(
                                out=U_s[mt][:, ch * 512:(ch + 1) * 512],
                                in_=pu[:], func=AF.Relu, bias=zero_col[:],
                                accum_out=apart2[:, ch, mt:mt + 1])
                        reduce_insts.append(red)

            # ---- every bulk DMA gated on the first U reduce so none of it
            # steals DMA-engine bandwidth from the critical c0b/c1b ---------
            gate = reduce_insts[0]
            bulk = [
                nc.sync.dma_start(consts[:], d_consts[:, :]),
                nc.gpsimd.dma_start(emb_s[0][:], d_emb[0:P, :]),
                nc.gpsimd.dma_start(emb_s[1][:], d_emb[P:T, :]),
                nc.sync.dma_start(late[0][:], d_late0[:, :]),
                nc.gpsimd.dma_start(late[1][:], d_late1[:, :]),
                nc.sync.dma_start(et_big[:, 0:4, :], d_ET[:, 0:4 * D]),
                nc.gpsimd.dma_start(et_big[:, 4:8, :], d_ET[:, 4 * D:8 * D]),
            ]
            for b in bulk:
                add_dep_helper(b.ins, gate.ins, sync=True,
                               reason="bulk DMA after crit DMA consumed")

            # ---- mask + iotaP accumulated into the two p-broadcast banks ---
            # pbm[st][s,t] = p_t - 1e30*[t < s + 128*st]  (+ cumsum terms below)
            # pbm banks share the ps256 pool slots: their lifetime (~10-17us)
            # ends before the G/AT/v tiles (~19us+) begin.
            pbm = [ps256.tile([P, T], F32, tag="ps", name=f"pbm{st}")
                   for st in range(2)]
            nc.tensor.matmul(pbm[0][:], tri_st, negI[0], start=True, stop=False)
            nc.tensor.matmul(pbm[1][:, 0:P], ones_t, negI[0][:, 0:P],
                             start=True, stop=True)
            nc.tensor.matmul(pbm[1][:, P:T], tri_st, negI[1][:, P:T],
                             start=True, stop=False)
            nc.tensor.matmul(pbm[0][:], ones2[:], pk_t, start=False,
                             stop=False)
            nc.tensor.matmul(pbm[1][:, P:T], ones2[:], pk_t[:, P:T],
                             start=False, stop=False)

            # filler matmuls: keep the PE HAM-warm while the logb chain runs
            for j in range(N_JUNK_MID):
                jp = psj.tile([P, 512], F32, tag="junkps", name=f"jmid{j}")
                nc.tensor.matmul(jp[:, 0:256], junk_sb[:, 0:P],
                                 junk_sb[:, 0:256], start=True, stop=True)

            # Critical chain (U sums -> logb -> hi/lo split -> cumsum matmuls
            # -> exp) gets top scheduler priority.
            with tc.high_priority():
                # b = a0 + a1 + 0.97*[t>0]; logb = ln(b), per-half so the
                # mt0 chain starts while the mt1 reduce is still running
                bcol = wp.tile([P, 2], F32, tag="bcol", name="bcol")
                lbcol = wp.tile([P, 2], F32, tag="lbcol", name="lbcol")
                for mt in range(2):
                    nc.vector.tensor_add(bcol[:, mt:mt + 1],
                                         apart2[:, 0, mt:mt + 1],
                                         apart2[:, 1, mt:mt + 1])
                    nc.vector.tensor_add(bcol[:, mt:mt + 1],
                                         bcol[:, mt:mt + 1],
                                         sc_t[:, mt, 0:1])
                    nc.scalar.activation(out=lbcol[:, mt:mt + 1],
                                         in_=bcol[:, mt:mt + 1], func=AF.Ln,
                                         bias=zero_col[:])

                # hi/lo split of logb, replicated across 128 columns.
                # mt0 chain entirely on scalar (ACT Identity broadcasts), mt1
                # entirely on vector, so both run in parallel after Ln.
                rr = {}
                lbl = wp.tile([P, 2], F32, tag="lbl", name="lbl")
                for mt, j in ((0, 0), (1, 0), (1, 1), (0, 1)):
                    if j == 0:
                        rh = wp.tile([P, P], BF16, tag=f"rrh{mt}",
                                     name=f"rrh{mt}")
                        if mt == 0:
                            nc.scalar.activation(
                                out=rh[:], in_=junk_sb[:, 0:P],
                                func=AF.Identity,
                                bias=lbcol[:, mt:mt + 1], scale=0.0)
                        else:
                            nc.vector.tensor_scalar(
                                rh[:], junk_sb[:, 0:P], 0.0,
                                lbcol[:, mt:mt + 1],
                                op0=ALU.mult, op1=ALU.add)
                        rr[(mt, 0)] = rh
                    else:
                        rh = rr[(mt, 0)]
                        rl = wp.tile([P, P], BF16, tag=f"rrl{mt}",
                                     name=f"rrl{mt}")
                        if mt == 0:
                            nc.scalar.activation(
                                out=lbl[:, 0:1], in_=rh[:, 0:1],
                                func=AF.Identity,
                                bias=lbcol[:, 0:1], scale=-1.0)
                            nc.scalar.activation(
                                out=rl[:], in_=junk_sb[:, 0:P],
                                func=AF.Identity,
                                bias=lbl[:, 0:1], scale=0.0)
                        else:
                            nc.vector.tensor_sub(lbl[:, 1:2],
                                                 lbcol[:, 1:2], rh[:, 0:1])
                            nc.vector.tensor_scalar(
                                rl[:], junk_sb[:, 0:P], 0.0,
                                lbl[:, 1:2], op0=ALU.mult, op1=ALU.add)
                        rr[(mt, 1)] = rl

                # column (strict) cumsum for q_s in PSUM (exact f32)
                qps = pss.tile([P, 2], F32, tag="pss", name="qps")
                for mt in range(2):
                    if mt == 0:
                        mms = [(tri_st, 0, 0), (tri_st, 0, 1)]
                    else:
                        mms = [(ones_t, 0, 0), (ones_t, 0, 1),
                               (tri_st, 1, 0), (tri_st, 1, 1)]
                    for i, (lhs, m2, j) in enumerate(mms):
                        nc.tensor.matmul(qps[:, mt:mt + 1], lhs,
                                         rr[(m2, j)][:, 0:1],
                                         start=(i == 0),
                                         stop=(i == len(mms) - 1))
                qsb = wp.tile([P, 2], F32, tag="qsb", name="qsb")
                for mt in range(2):
                    nc.vector.tensor_add(qsb[:, mt:mt + 1], qps[:, mt:mt + 1],
                                         sc_t[:, mt, 2:3])

                # p-broadcast cumsum into both masked banks; negation is baked
                # into cumA (host sends -1/0 inclusive-cumsum blocks).
                ct_exps = []
                ct2 = wp.tile([P, 2, T], BF16, tag="CT", name="CT")
                for st in range(2):
                    cols = slice(0, T) if st == 0 else slice(P, T)
                    for i, (mt, j) in enumerate(
                            ((0, 0), (1, 0), (1, 1), (0, 1))):
                        nc.tensor.matmul(pbm[st][:, cols], rr[(mt, j)][:],
                                         cumA[mt][:, cols], start=False,
                                         stop=(i == 3))
                    # CT[s,t] = exp(q_s + p_t + mask) straight from PSUM
                    exp_i = nc.scalar.activation(out=ct2[:, st, :],
                                                 in_=pbm[st][:],
                                                 func=AF.Exp,
                                                 bias=qsb[:, st:st + 1],
                                                 scale=1.0)
                    ct_exps.append(exp_i)
                CT_s = [ct2[:, 0, :], ct2[:, 1, :]]

            # ---- X^T = U^T C^T  (n on partitions, T free), 512-wide pairs --
            XT_p = []
            for jp in range(4):
                px = ps512.tile([P, 512], F32, tag="pu", name="px")
                for h in range(2):
                    m = 2 * jp + h
                    nc.tensor.matmul(px[:, h * T:(h + 1) * T],
                                     U_s[0][:, m * P:(m + 1) * P],
                                     CT_s[0][:], start=True, stop=False)
                    nc.tensor.matmul(px[:, h * T + P:(h + 1) * T],
                                     U_s[1][:, m * P:(m + 1) * P],
                                     CT_s[1][:, P:T], start=False, stop=True)
                xt = wp.tile([P, 512], BF16, tag=f"XT{jp}", name=f"XT{jp}")
                if jp % 2 == 0:
                    nc.vector.tensor_copy(xt[:], px[:])
                else:
                    nc.scalar.copy(xt[:], px[:])
                XT_p.append(xt)

            def xs(k, lo, hi):  # slice [lo:hi] of n-chunk k from the pairs
                return XT_p[k // 2][:, (k % 2) * T + lo:(k % 2) * T + hi]

            # ---- W = ln(emb rows) -------------------------------------------
            # W chain is slack work (needed only by A^T at ~22us); hard-order
            # every op after the CT Exps so the scheduler cannot wedge it
            # into critical scalar/vector slots during the logb chain.
            W_s = []
            for mt in range(2):
                st6 = wp.tile([P, 6], F32, tag=f"wst{mt}", name=f"wst{mt}")
                bn = nc.vector.bn_stats(st6[:], emb_s[mt][:])
                for dep in reduce_insts + ct_exps:
                    add_dep_helper(bn.ins, dep.ins, sync=False,
                                   reason="critical chain before W stats")
                mv = wp.tile([P, 2], F32, tag=f"wmv{mt}", name=f"wmv{mt}")
                nc.vector.bn_aggr(mv[:], st6[:])
                lv = wp.tile([P, 1], F32, tag=f"wlv{mt}", name=f"wlv{mt}")
                wln = nc.scalar.activation(out=lv[:], in_=mv[:, 1:2],
                                           func=AF.Ln, bias=eps_col[:])
                for dep in ct_exps:
                    add_dep_helper(wln.ins, dep.ins, sync=False,
                                   reason="CT exps before W Ln on scalar")
                rs = wp.tile([P, 1], F32, tag=f"wrs{mt}", name=f"wrs{mt}")
                nc.scalar.activation(out=rs[:], in_=lv[:], func=AF.Exp,
                                     bias=zero_col[:], scale=-0.5)
                w = wp.tile([P, D], BF16, tag=f"W{mt}", name=f"W{mt}")
                nc.vector.tensor_scalar(w[:], emb_s[mt][:], mv[:, 0:1],
                                        rs[:], op0=ALU.subtract, op1=ALU.mult)
                W_s.append(w)

            # ---- G = X X^T ; GD = G o Dup -----------------------------------
            # DupT[s,t] = decay^(t-1-s) for s<t else 0, so the st=1 row block
            # only needs columns t >= 128; its left half is identically zero.
            GD_s = []
            for st in range(2):
                cols = slice(0, T) if st == 0 else slice(P, T)
                pg = ps256.tile([P, T], F32, tag="ps", name="pg")
                for k in range(8):
                    nc.tensor.matmul(pg[:, cols],
                                     xs(k, st * P, (st + 1) * P),
                                     xs(k, cols.start, cols.stop),
                                     start=(k == 0), stop=(k == 7))
                gd = wp.tile([P, T], BF16, tag=f"GD{st}", name=f"GD{st}")
                if st == 1:
                    nc.gpsimd.memset(gd[:, 0:P], 0.0)
                nc.vector.tensor_mul(gd[:, cols], pg[:, cols],
                                     DupT_s[st][:, cols])
                GD_s.append(gd)

            # ---- A^T = W^T (G o D)  ([d, t]), no layernorm needed -----------
            AT_s = []
            for dt_ in range(2):
                pa = ps256.tile([P, T], F32, tag="ps", name="pa")
                for k in range(2):
                    nc.tensor.matmul(pa[:], W_s[k][:, dt_ * P:(dt_ + 1) * P],
                                     GD_s[k][:], start=(k == 0), stop=(k == 1))
                at = wp.tile([P, T], BF16, tag=f"AT{dt_}", name=f"AT{dt_}")
                if dt_ == 0:
                    nc.vector.tensor_copy(at[:], pa[:])
                else:
                    nc.scalar.copy(at[:], pa[:])
                AT_s.append(at)

            # ---- va_t = sum_d a*[t,d]^2 -> per-row eps for the final LN ----
            # (exact compensation for the dropped 1/sigma of ln(a*):
            #  eps_t = LN_EPS * (va_t/D + LN_EPS))
            sq_s = []
            for k in range(2):
                sq = wp.tile([P, T], BF16, tag=f"sq{k}", name=f"sq{k}")
                nc.gpsimd.tensor_mul(sq[:], AT_s[k][:], AT_s[k][:])
                sq_s.append(sq)
            epsva = []
            for mt in range(2):
                vap = pss.tile([P, 1], F32, tag="pss", name=f"va{mt}")
                for k in range(2):
                    nc.tensor.matmul(vap[:], sq_s[k][:, mt * P:(mt + 1) * P],
                                     onescol[:], start=(k == 0), stop=(k == 1))
                ev = wp.tile([P, 1], F32, tag=f"ev{mt}", name=f"ev{mt}")
                nc.vector.tensor_scalar(ev[:], vap[:], LN_EPS / D,
                                        LN_EPS * LN_EPS,
                                        op0=ALU.mult, op1=ALU.add)
                epsva.append(ev)

            # ---- y^T = relu(Dy A^T) o X^T, 512-wide pairs -------------------
            yT_p = []
            for jp in range(4):
                py = ps512.tile([P, 512], F32, tag="pu", name="py")
                for h in range(2):
                    m = 2 * jp + h
                    for k in range(2):
                        nc.tensor.matmul(py[:, h * T:(h + 1) * T],
                                         DyT_s[k][:, m * P:(m + 1) * P],
                                         AT_s[k][:], start=(k == 0),
                                         stop=(k == 1))
                yt = wp.tile([P, 512], BF16, tag=f"yT{jp}", name=f"yT{jp}")
                if jp % 2 == 0:
                    nc.vector.scalar_tensor_tensor(
                        out=yt[:], in0=py[:], scalar=0.0, in1=XT_p[jp][:],
                        op0=ALU.max, op1=ALU.mult)
                else:
                    yr = wp.tile([P, 512], BF16, tag=f"yR{jp}",
                                 name=f"yR{jp}")
                    nc.scalar.activation(out=yr[:], in_=py[:], func=AF.Relu,
                                         bias=zero_col[:])
                    nc.vector.tensor_mul(yt[:], yr[:], XT_p[jp][:])
                yT_p.append(yt)

            def ys(k, lo, hi):
                return yT_p[k // 2][:, (k % 2) * T + lo:(k % 2) * T + hi]

            # ---- v = y E^T ([t, d]) + layernorm (per-row eps) + store -------
            for mt in range(2):
                pv = ps256.tile([P, D], F32, tag="ps", name="pv")
                for k in range(8):
                    nc.tensor.matmul(pv[:], ys(k, mt * P, (mt + 1) * P),
                                     ET_s[k][:], start=(k == 0), stop=(k == 7))
                st6 = wp.tile([P, 6], F32, tag=f"ost{mt}", name=f"ost{mt}")
                nc.vector.bn_stats(st6[:], pv[:])
                mv = wp.tile([P, 2], F32, tag=f"omv{mt}", name=f"omv{mt}")
                nc.vector.bn_aggr(mv[:], st6[:])
                lv = wp.tile([P, 1], F32, tag=f"olv{mt}", name=f"olv{mt}")
                nc.scalar.activation(out=lv[:], in_=mv[:, 1:2], func=AF.Ln,
                                     bias=epsva[mt][:])
                rs = wp.tile([P, 1], F32, tag=f"ors{mt}", name=f"ors{mt}")
                nc.scalar.activation(out=rs[:], in_=lv[:], func=AF.Exp,
                                     bias=zero_col[:], scale=-0.5)
                ov = wp.tile([P, D], BF16, tag=f"ov{mt}", name=f"ov{mt}")
                nc.vector.tensor_scalar(ov[:], pv[:], mv[:, 0:1], rs[:],
                                        op0=ALU.subtract, op1=ALU.mult)
                eng = nc.sync if mt == 0 else nc.scalar
                eng.dma_start(d_out[mt * P:(mt + 1) * P, :], ov[:])

    nc.finalize()
    return nc


_NC_CACHE = {}


def _get_nc(_unused=True):
    if "nc" not in _NC_CACHE:
        _NC_CACHE["nc"] = _build_nc()
    return _NC_CACHE["nc"]


def _host_consts():
    bf = ml_dtypes.bfloat16
    ii = np.arange(T, dtype=np.float64)
    ln097 = np.log(np.float64(DECAY))
    DupT = np.where(
        ii[:, None] < ii[None, :],
        np.float64(DECAY) ** (ii[None, :] - 1 - ii[:, None]),
        0.0,
    ).astype(np.float32)
    sc = np.zeros((T, 4), np.float32)
    sc[:, 0] = DECAY
    sc[0, 0] = 0.0
    sc[:, 1] = (ii * ln097).astype(np.float32)
    sc[:, 2] = (-ii * ln097).astype(np.float32)

    tri_strict = np.triu(np.ones((P, P), np.float32), k=1)
    ones = np.ones((P, P), np.float32)
    incl = np.triu(np.ones((P, P), np.float32), k=0)
    zeros = np.zeros((P, P), np.float32)
    # cumA carries the NEGATIVE inclusive-cumsum blocks (the p_t term is
    # -sum_{r<=t} logb_r, and the rr broadcasts are positive).
    cumA0 = np.concatenate([-incl, -ones], axis=1)
    cumA1 = np.concatenate([zeros, -incl], axis=1)
    negI0 = np.concatenate([NEGBIG * np.eye(P, dtype=np.float32), zeros],
                           axis=1)
    negI1 = np.concatenate([zeros, NEGBIG * np.eye(P, dtype=np.float32)],
                           axis=1)
    iotaP = (ii * ln097).astype(np.float32)
    p_hi = iotaP.astype(np.float32).astype(bf)
    p_lo = (iotaP - p_hi.astype(np.float32)).astype(bf)
    pkpad = np.zeros((P, T), np.float32)
    pkpad[0] = p_hi.astype(np.float32)
    pkpad[1] = p_lo.astype(np.float32)
    consts = np.concatenate(
        [tri_strict, ones, cumA0, cumA1, negI0, negI1, pkpad],
        axis=1).astype(bf)
    return sc, consts, DupT.astype(bf)


def make_in_maps(embeddings, E, Dx, Dy):
    bf = ml_dtypes.bfloat16
    emb = np.ascontiguousarray(np.asarray(embeddings, dtype=np.float32))
    E = np.asarray(E, dtype=np.float32)
    Dx = np.asarray(Dx, dtype=np.float32)
    Dy = np.asarray(Dy, dtype=np.float32)
    sc, consts, DupT_bf = _host_consts()

    DxT = np.ascontiguousarray(Dx.T).astype(bf)      # [D, N]
    DyT = np.ascontiguousarray(Dy.T).astype(bf)      # [D, N]
    ETp = np.ascontiguousarray(                      # [P, 8*D]
        E.T.reshape(8, P, D).transpose(1, 0, 2).reshape(P, 8 * D)).astype(bf)

    shared = {
        "sc": sc, "consts": consts, "ET": ETp,
        "late0": np.ascontiguousarray(
            np.concatenate([DupT_bf[0:P], DyT[0:P]], axis=1)),
        "late1": np.ascontiguousarray(
            np.concatenate([DupT_bf[P:T], DyT[P:T]], axis=1)),
    }

    in_maps = []
    for b in range(B):
        embT_bf = np.ascontiguousarray(emb[b].T).astype(bf)  # [D, T]
        m = dict(shared)
        m["c0a"] = np.ascontiguousarray(
            np.concatenate([embT_bf[0:P], DxT[0:P, 0:512]], axis=1))
        m["c0b"] = np.ascontiguousarray(DxT[0:P, 512:1024])
        m["c1a"] = np.ascontiguousarray(
            np.concatenate([embT_bf[P:D], DxT[P:D, 0:512]], axis=1))
        m["c1b"] = np.ascontiguousarray(DxT[P:D, 512:1024])
        m["emb"] = emb[b].astype(bf)
        in_maps.append(m)
    return in_maps


def kernel(embeddings, E, Dx, Dy, **_kw):
    in_maps = make_in_maps(embeddings, E, Dx, Dy)
    nc = _get_nc()
    res = run_bass_kernel_spmd(nc, in_maps, core_ids=list(range(B)))
    return np.stack([np.asarray(r["out"]).astype(np.float32)
                     for r in res.results], axis=0)
